# revision 2
# baseline (speedup 1.0000x reference)
"""DeepseekV2 MoE gate (noaux_tc sigmoid routing) on 8 Trainium2 cores, v2.

Strategy
--------
Token-parallel SPMD: each core routes a 1024-token slice.  All data
marshaling that doesn't need the device happens on the host: x is
transposed, split into bf16 hi/lo, and pre-tiled so every device DMA is
a long contiguous run per partition.

Device program per core (instruction-count-minimized):
  - logits^T = W @ x^T computed with W chunks stationary and 512-token
    moving operands, in plain fp32 (the PE's self-loading fp32 matmult):
    56 k-tiles x 2 expert-halves x 2 token batches = 224 matmuls into
    four [128e, 512t] PSUM accumulators.  fp32 operands make the logits
    exact (no hi/lo split, no selection flips) and halve the PE
    instruction count again: fp32 matmults carry their own weight load.
  - 16 PE transposes restore [token, expert] tiles; ACT applies sigmoid.
  - noaux_tc group-limited top-8 runs as a stage-major chain across the
    8 token tiles: every DVE instruction issues back-to-back with its
    dependency 8 instructions behind it, so the engine never stalls on
    the ~us semaphore latency of a serial per-tile chain.
  - top-k weights are gathered from the unbiased scores with an
    index-match (one-hot vs iota) instead of the value-mark/match dance.
"""

import numpy as np

P = 128
TOKENS, HIDDEN, NEXP = 8192, 7168, 256
NCORES = 8
T_CORE = TOKENS // NCORES
TOP_K = 8
N_GROUP = 8
TOPK_GROUP = 4
ROUTED_SCALE = 2.5
NEG_INF = -1.0e9

KT = HIDDEN // P          # 56 contraction k-tiles
NB = 2                    # token batches per core
TB = T_CORE // NB         # 512 tokens per batch
NTT = T_CORE // P         # 8 token tiles per core
KC = 8                    # k-tiles per x DMA chunk
NKC = KT // KC            # 7 chunks per batch


def build_program(repeat=1, legalize=True):
    from contextlib import ExitStack

    import concourse.bass as bass
    import concourse.mybir as mybir
    from concourse.masks import make_identity
    from concourse.tile import TileContext

    f32 = mybir.dt.float32
    bf16 = mybir.dt.bfloat16
    i32 = mybir.dt.int32
    u32 = mybir.dt.uint32
    AO = mybir.AluOpType
    AX = mybir.AxisListType

    nc = bass.Bass()
    xt_d = nc.declare_dram_parameter("xt", [P, NB * KT * TB], f32, isOutput=False)
    wt_d = nc.declare_dram_parameter("wt", [P, KT * NEXP], f32, isOutput=False)
    bias_d = nc.declare_dram_parameter("bias", [NEXP], f32, isOutput=False)
    oi_d = nc.declare_dram_parameter("topk_idx", [T_CORE, TOP_K], i32, isOutput=True)
    ow_d = nc.declare_dram_parameter("topk_w", [T_CORE, TOP_K], f32, isOutput=True)

    with TileContext(nc) as tc, ExitStack() as ctx:
        consts = ctx.enter_context(tc.tile_pool(name="consts", bufs=1))
        wpool = ctx.enter_context(tc.tile_pool(name="wpool", bufs=1))
        xpool = ctx.enter_context(tc.tile_pool(name="xpool", bufs=2))
        lgp = ctx.enter_context(tc.tile_pool(name="lgp", bufs=2, space="PSUM"))
        lgs = ctx.enter_context(tc.tile_pool(name="lgs", bufs=2))
        stp = ctx.enter_context(tc.tile_pool(name="stp", bufs=2, space="PSUM"))
        warmp = ctx.enter_context(tc.tile_pool(name="warmp", bufs=1, space="PSUM"))
        tk = ctx.enter_context(tc.tile_pool(name="tk", bufs=1))
        big = ctx.enter_context(tc.tile_pool(name="big", bufs=8))
        ohp = ctx.enter_context(tc.tile_pool(name="ohp", bufs=4))
        outp = ctx.enter_context(tc.tile_pool(name="outp", bufs=8))

        # ---- constants ----
        ident = consts.tile([P, P], f32)
        make_identity(nc, ident)

        bias_b = consts.tile([P, NEXP], f32)
        bias_ap = bass.AP(
            tensor=bias_d.tensor if hasattr(bias_d, "tensor") else bias_d,
            offset=0,
            ap=[[0, P], [1, NEXP]],
        )
        nc.gpsimd.dma_start(out=bias_b, in_=bias_ap)

        iota_i = consts.tile([P, NEXP], i32)
        nc.gpsimd.iota(iota_i, pattern=[[1, NEXP]], base=0, channel_multiplier=0)
        iota_f = consts.tile([P, NEXP], f32)
        nc.vector.tensor_copy(iota_f, iota_i)

        wt = wpool.tile([P, KT * NEXP], f32)
        nc.sync.dma_start(out=wt, in_=wt_d[:, :])
        w3 = wt.rearrange("p (k e) -> p k e", e=NEXP)

        # Wait-absorbers: walrus allows only one sync-wait on the LDW half of
        # a Matmult; give PE cheap ops that consume the setup dependencies.
        wrm = warmp.tile([P, P], f32, tag="warm")
        nc.tensor.transpose(wrm[:, 0:P], ident, ident)
        wmm = warmp.tile([P, P], f32, tag="warm")
        nc.tensor.matmul(wmm[:, 0:1], lhsT=wt[:, 0:P], rhs=wt[:, 0:1],
                         start=True, stop=True)

        def emit_body():
            scores = []
            for b in range(NB):
                lg = [lgp.tile([P, TB], f32, tag=f"lg{h}", name=f"lg{h}") for h in range(2)]
                for c in range(NKC):
                    off = (b * KT + c * KC) * TB
                    xc = xpool.tile([P, KC * TB], f32, tag="xc")
                    nc.sync.dma_start(out=xc, in_=xt_d[:, off:off + KC * TB])
                    xwarm = warmp.tile([P, P], f32, tag="warm")
                    nc.tensor.matmul(xwarm[:, 0:1], lhsT=xc[:, 0:P], rhs=xc[:, 0:1],
                                     start=True, stop=True)
                    for kk in range(KC):
                        k = c * KC + kk
                        x_k = xc[:, kk * TB:(kk + 1) * TB]
                        for h in range(2):
                            w_k = w3[:, k, h * P:(h + 1) * P]
                            nc.tensor.matmul(lg[h], lhsT=w_k, rhs=x_k,
                                             start=(k == 0), stop=(k == KT - 1))

                lgb = [lgs.tile([P, TB], f32, tag=f"lgs{h}", name=f"lgs{h}") for h in range(2)]
                for h in range(2):
                    nc.scalar.copy(out=lgb[h], in_=lg[h])
                for j in range(TB // P):
                    st = stp.tile([P, NEXP], f32)
                    for h in range(2):
                        nc.tensor.transpose(st[:, h * P:(h + 1) * P],
                                            lgb[h][:, j * P:(j + 1) * P], ident)
                    sc = tk.tile([P, NEXP], f32, tag=f"scores{b * 4 + j}")
                    nc.scalar.activation(sc, st, mybir.ActivationFunctionType.Sigmoid)
                    scores.append(sc)

            # ---- stage-major noaux_tc top-8 across the 8 token tiles ----
            T = NTT
            sfc = [tk.tile([P, NEXP], f32, tag=f"sfc{t}", name=f"sfc{t}") for t in range(T)]
            for t in range(T):
                nc.vector.tensor_add(sfc[t], scores[t], bias_b)
            g1 = [tk.tile([P, N_GROUP], f32, tag=f"g1{t}", name=f"g1{t}") for t in range(T)]
            for t in range(T):
                nc.vector.tensor_reduce(g1[t], sfc[t].rearrange("p (g e) -> p g e", g=N_GROUP),
                                        axis=AX.X, op=AO.max)
            rep = [big.tile([P, NEXP], f32, tag="rep", name=f"rep{t}") for t in range(T)]
            for t in range(T):
                nc.vector.match_replace(out=rep[t], in_to_replace=g1[t],
                                        in_values=sfc[t], imm_value=NEG_INF)
            gs = [tk.tile([P, N_GROUP], f32, tag=f"gs{t}", name=f"gs{t}") for t in range(T)]
            for t in range(T):
                # g2 then gs fused: reduce into gs, then add g1
                nc.vector.tensor_reduce(gs[t], rep[t].rearrange("p (g e) -> p g e", g=N_GROUP),
                                        axis=AX.X, op=AO.max)
            for t in range(T):
                nc.vector.tensor_add(gs[t], gs[t], g1[t])
            g8 = [tk.tile([P, 8], f32, tag=f"g8{t}", name=f"g8{t}") for t in range(T)]
            for t in range(T):
                nc.vector.max(out=g8[t], in_=gs[t])
            pen = [tk.tile([P, N_GROUP], f32, tag=f"pen{t}", name=f"pen{t}") for t in range(T)]
            for t in range(T):
                nc.vector.tensor_scalar(pen[t], gs[t],
                                        g8[t][:, TOPK_GROUP - 1:TOPK_GROUP],
                                        NEG_INF, op0=AO.is_lt, op1=AO.mult)
            masked = [big.tile([P, NEXP], f32, tag="masked", name=f"masked{t}") for t in range(T)]
            for t in range(T):
                nc.vector.tensor_tensor(
                    masked[t].rearrange("p (g e) -> p g e", g=N_GROUP),
                    sfc[t].rearrange("p (g e) -> p g e", g=N_GROUP),
                    pen[t].rearrange("p (g o) -> p g o", o=1)
                        .to_broadcast([P, N_GROUP, NEXP // N_GROUP]),
                    op=AO.add)
            top8 = [tk.tile([P, 8], f32, tag=f"top8{t}", name=f"top8{t}") for t in range(T)]
            for t in range(T):
                nc.vector.max(out=top8[t], in_=masked[t])
            idxu = [tk.tile([P, 8], u32, tag=f"idxu{t}", name=f"idxu{t}") for t in range(T)]
            for t in range(T):
                nc.vector.max_index(idxu[t], top8[t], masked[t])
            idxf = [tk.tile([P, 8], f32, tag=f"idxf{t}", name=f"idxf{t}") for t in range(T)]
            for t in range(T):
                nc.vector.tensor_copy(idxf[t], idxu[t])
            # gather unbiased scores at idxu via index match; two rounds of 4
            # tiles so only 4 of the wide one-hot tiles are ever alive
            ssel = [tk.tile([P, 8], f32, tag=f"ssel{t}", name=f"ssel{t}") for t in range(T)]
            for r in range(0, T, 4):
                tiles = range(r, r + 4)
                oh = {t: ohp.tile([P, 8 * NEXP], f32, tag="oh", name=f"oh{t}") for t in tiles}
                for t in tiles:
                    nc.vector.tensor_tensor(
                        oh[t].rearrange("p (a e) -> p a e", a=8),
                        idxf[t].rearrange("p (a o) -> p a o", o=1)
                            .to_broadcast([P, 8, NEXP]),
                        iota_f.rearrange("p (o e) -> p o e", o=1)
                            .to_broadcast([P, 8, NEXP]),
                        op=AO.is_equal)
                for t in tiles:
                    nc.vector.tensor_tensor(
                        oh[t].rearrange("p (a e) -> p a e", a=8),
                        oh[t].rearrange("p (a e) -> p a e", a=8),
                        scores[t].rearrange("p (o e) -> p o e", o=1)
                            .to_broadcast([P, 8, NEXP]),
                        op=AO.mult)
                for t in tiles:
                    nc.vector.tensor_reduce(ssel[t], oh[t].rearrange("p (a e) -> p a e", a=8),
                                            axis=AX.X, op=AO.add)
            ssum = [tk.tile([P, 1], f32, tag=f"ssum{t}", name=f"ssum{t}") for t in range(T)]
            for t in range(T):
                nc.vector.tensor_reduce(ssum[t], ssel[t], axis=AX.X, op=AO.add)
            rcp = [tk.tile([P, 1], f32, tag=f"rcp{t}", name=f"rcp{t}") for t in range(T)]
            for t in range(T):
                nc.vector.reciprocal(rcp[t], ssum[t])
            wfin = [outp.tile([P, 8], f32, tag="wfin", name=f"wfin{t}") for t in range(T)]
            for t in range(T):
                nc.vector.tensor_scalar(wfin[t], ssel[t], rcp[t], ROUTED_SCALE,
                                        op0=AO.mult, op1=AO.mult)
            idxi = [outp.tile([P, 8], i32, tag="idxi", name=f"idxi{t}") for t in range(T)]
            for t in range(T):
                nc.vector.tensor_copy(idxi[t], idxu[t])
            for t in range(T):
                nc.sync.dma_start(out=oi_d[t * P:(t + 1) * P, :], in_=idxi[t])
                nc.sync.dma_start(out=ow_d[t * P:(t + 1) * P, :], in_=wfin[t])

        if repeat > 1:
            with tc.For_i(0, repeat, 1):
                emit_body()
        else:
            emit_body()

    if legalize:
        _legalize_waits(nc)
    return nc


_WAIT_SPLIT_SKIP = {"InstEventSemaphore", "InstUnconditionalBranch",
                    "InstCall", "InstRegisterMove", "InstConditionalBranch"}


def _legalize_waits(nc):
    """Walrus codegen allows a single sync-wait on most TPB instruction
    structs; hoist extra waits into standalone EventSemaphore instructions
    executed just before the offending instruction on the same engine."""
    import concourse.mybir as mybir

    for blk in nc.m.functions[0].blocks:
        out = []
        changed = False
        for inst in blk.instructions:
            si = getattr(inst, "sync_info", None)
            if (si is not None and len(si.on_wait) > 1
                    and type(inst).__name__ not in _WAIT_SPLIT_SKIP):
                waits = list(si.on_wait)
                for j, w in enumerate(waits[:-1]):
                    es = mybir.InstEventSemaphore(
                        name=f"{inst.name}-xw{j}", ins=[], outs=[])
                    es.engine = inst.engine
                    es.sync_info = mybir.SyncInfo(on_wait=[w], on_update=[])
                    out.append(es)
                inst.sync_info = mybir.SyncInfo(
                    on_wait=[waits[-1]], on_update=list(si.on_update))
                changed = True
            out.append(inst)
        if changed:
            blk.instructions = out


def _host_prep_w(weight):
    w32 = np.asarray(weight, dtype=np.float32)
    kt = w32.shape[1] // P
    # [e, (k p)] -> [p][k][e] pre-tiled so the device DMA is contiguous
    return np.ascontiguousarray(
        w32.T.reshape(kt, P, -1).transpose(1, 0, 2).reshape(P, -1))


def _host_prep_x(x):
    """x [TOKENS, HIDDEN] f32 -> per-core pre-tiled fp32:
    A[c][p, ((b*KT + k)*TB + t)] = x[c*T_CORE + b*TB + t, k*P + p]."""
    v = x.reshape(NCORES, NB, TB, KT, P)       # [c, b, t, k, p]
    v = v.transpose(0, 4, 1, 3, 2)             # [c, p, b, k, t]
    return np.ascontiguousarray(v).reshape(NCORES, P, NB * KT * TB)


_CACHED_NC = None


def kernel(hidden_states, weight, e_score_correction_bias):
    global _CACHED_NC
    from concourse.bass_utils import run_bass_kernel_spmd

    x = np.asarray(hidden_states, dtype=np.float32)
    b = np.asarray(e_score_correction_bias, dtype=np.float32)
    wt = _host_prep_w(weight)
    xt = _host_prep_x(x)

    if _CACHED_NC is None:
        _CACHED_NC = build_program()
    nc = _CACHED_NC

    in_maps = []
    for c in range(NCORES):
        in_maps.append({
            "xt": xt[c],
            "wt": wt,
            "bias": b,
        })
    res = run_bass_kernel_spmd(nc, in_maps, core_ids=list(range(NCORES)))
    idx = np.concatenate([r["topk_idx"] for r in res.results], axis=0)
    w = np.concatenate([r["topk_w"] for r in res.results], axis=0)
    return idx.astype(np.int32), w.astype(np.float32)
